# revision 35
# baseline (speedup 1.0000x reference)
"""Trainium2 Bass kernel for Disk descriptor mutual-NN matching (retrieval_knn).

Strategy (8 NeuronCores, shard descriptors1 columns M across cores):
  - Each core c holds full d0 [256, 8192] and its d1 shard [256, 1024].
  - S_c = d0.T @ d1_c via f16x3 split matmuls (3 f16 passes = 3/4 the fp32
    PE time; products exact below fp32 accumulation noise).
  - Forward: per-row top-8 values over the core's 1024 columns (DVE
    InstMax).  No index pass: indices are recovered on the host (below).
  - Backward: per-lane running elementwise max over the 64 row-chunks
    (DVE tensor_max fold, 1 op/chunk) -> fold[p, j] = max over rows
    {p + 128k} of column j.  The [128, 1024] fold state is DMAed out and
    the host reduces over the 128 lanes: exact column max M1[j]
    (bit-identical to the forward S values), its lane, and a cross-lane
    runner-up M2x[j] (== true second unless the column's top-2 share a
    lane, which only matters on exact value ties -- where the reference
    fails the ratio test anyway; with rt=1.0 the ratio test is a pure
    tie detector).
  - Host mutual test: row i is matched to column j iff its top value
    s1[i] IS the column max: s1[i] == M1[j], exact because both sides
    are the same f32 bits from the same PSUM drain.  This also recovers
    the matched column INDEX by value search (unique absent f32 bit
    collisions between distinct dot products, which are detected).
"""

import sys

if "/opt/trn_rl_repo" not in sys.path:
    sys.path.insert(0, "/opt/trn_rl_repo")

import numpy as np

N_KPTS = 8192
M_KPTS = 8192
F_DIM = 256
N_CORES = 8
M_SHARD = M_KPTS // N_CORES  # 1024

SQRT_2 = np.float32(1.414213)
CLIP_LO = np.float32(1e-6)
ONE = np.float32(1.0)

DTYPE_MODE = "f16x3"


def _split_f16(a32):
    """f32 -> (h, h/32, 32*(a-h)) as float16, with f16-subnormal highs
    flushed into the residual so no information rides on f16 subnormals."""
    h = a32.astype(np.float16)
    h[np.abs(a32) < 6.104e-5] = np.float16(0)
    l = a32 - h.astype(np.float32)
    h_s = (h.astype(np.float32) / 32.0).astype(np.float16)
    l_s = (l * 32.0).astype(np.float16)
    return h, h_s, l_s


# --------------------------------------------------------------------------
# Device kernel builder
# --------------------------------------------------------------------------

def build_kernel(n_rows=N_KPTS, m_shard=M_SHARD, f_dim=F_DIM,
                 dtype_mode=DTYPE_MODE):
    """Build the per-core SPMD Bass program.

    Inputs (per core, f16 split terms):
      d0h/d0hs/d0ls: [kf, 128, n_rows]   (descriptors0, K-chunked)
      d1h/d1hs/d1ls: [kf, 128, m_shard]  (this core's descriptors1 shard)
    Outputs (per core):
      fwd_val [128, n_chunks*8] f32  (row top-8 per 128-row chunk)
      fold    [128, m_shard] f32     (per-lane column maxes)
    """
    import concourse.bacc as bacc
    import concourse.mybir as mybir
    import concourse.tile as tile

    kf = f_dim // 128
    n_chunks = n_rows // 128          # row chunks
    m_tiles = max(1, m_shard // 512)  # 512-wide column tiles (PSUM banks)
    mw = min(512, m_shard)
    assert m_shard % 128 == 0 and f_dim % 128 == 0

    nc = bacc.Bacc("TRN2", target_bir_lowering=False, debug=False,
                   num_devices=1)

    if dtype_mode == "fp32":
        in_names = ["d0", "d1"]
        in_dt = mybir.dt.float32
    else:
        in_names = ["d0h", "d0hs", "d0ls", "d1h", "d1hs", "d1ls"]
        in_dt = mybir.dt.float16
    in_dram = {}
    for nm in in_names:
        nw = n_rows if nm.startswith("d0") else m_shard
        in_dram[nm] = nc.dram_tensor(nm, [kf, 128, nw], in_dt,
                                     kind="ExternalInput")
    fwd_val = nc.dram_tensor("fwd_val", [128, n_chunks * 8], mybir.dt.float32,
                             kind="ExternalOutput")
    fold_out = nc.dram_tensor("fold", [128, m_shard], mybir.dt.float32,
                              kind="ExternalOutput")
    # raw S rows of chunk n_chunks-2; its forward top-8 runs on the host
    # so the post-matmul device work is only the final folds
    s_tail = nc.dram_tensor("s_tail", [128, m_shard], mybir.dt.float32,
                            kind="ExternalOutput")

    with tile.TileContext(nc) as tc:
        with tc.tile_pool(name="persist", bufs=1) as persist, \
             tc.tile_pool(name="schunk", bufs=4) as schunk_pool, \
             tc.tile_pool(name="outs", bufs=1) as outs_pool, \
             tc.tile_pool(name="psf", bufs=3, space="PSUM") as psf:

            # resident inputs; d0 loads split along n so early fwd units
            # unblock before the full load completes
            in_sb = {}
            for nm in in_names:
                nw = n_rows if nm.startswith("d0") else m_shard
                in_sb[nm] = [persist.tile([128, nw], in_dt,
                                          name=f"{nm}sb{k}", tag=f"{nm}sb{k}")
                             for k in range(kf)]
            # input loads: the 12 tiles chunk 0 consumes are co-scheduled
            # by first-use time across the SP / ACT / Pool sequencers so
            # none of them serializes behind SP's 565 ns issue slot;
            # the remaining d0 row-pieces stream on SP well ahead of use
            n_split = 8 if n_rows % 1024 == 0 else 1
            d0n = [nm for nm in in_names if nm.startswith("d0")]
            p0 = slice(0, n_rows // n_split)
            if dtype_mode == "fp32":
                for k in range(kf):
                    nc.sync.dma_start(in_sb["d0"][k][:, p0],
                                      in_dram["d0"][k][:, p0])
                    nc.scalar.dma_start(in_sb["d1"][k][:], in_dram["d1"][k])
            else:
                for k in range(kf):
                    nc.sync.dma_start(in_sb["d0h"][k][:, p0],
                                      in_dram["d0h"][k][:, p0])
                    nc.sync.dma_start(in_sb["d0hs"][k][:, p0],
                                      in_dram["d0hs"][k][:, p0])
                for k in range(kf):
                    nc.scalar.dma_start(in_sb["d1h"][k][:], in_dram["d1h"][k])
                    nc.scalar.dma_start(in_sb["d1ls"][k][:],
                                        in_dram["d1ls"][k])
                    nc.scalar.dma_start(in_sb["d0ls"][k][:, p0],
                                        in_dram["d0ls"][k][:, p0])
                for k in range(kf):
                    nc.gpsimd.dma_start(in_sb["d1hs"][k][:],
                                        in_dram["d1hs"][k])
            for p in range(1, n_split):
                sl = slice(p * n_rows // n_split, (p + 1) * n_rows // n_split)
                for k in range(kf):
                    for nm in d0n:
                        nc.sync.dma_start(in_sb[nm][k][:, sl],
                                          in_dram[nm][k][:, sl])
            if dtype_mode == "fp32":
                terms = [("d0", "d1")]
            else:
                terms = [("d0h", "d1h"), ("d0hs", "d1ls"), ("d0ls", "d1hs")]

            zeros = persist.tile([128, 128], mybir.dt.float32, name="zeros")
            nc.vector.memset(zeros[:], 0)
            # warm-up matmul: starts the PE p-state ramp clock while input
            # DMAs are still streaming (zeros need no DMA)
            warm = psf.tile([128, 8], mybir.dt.float32, tag="pf", name="warm",
                            padded_shape=[128, m_tiles * mw])
            nc.tensor.matmul(warm[:], zeros[:], zeros[:, :8],
                             start=True, stop=True)

            fv_sb = outs_pool.tile([128, n_chunks * 8], mybir.dt.float32)
            # per-lane running column max over row chunks
            fold = outs_pool.tile([128, m_shard], mybir.dt.float32,
                                  name="fold")

            n_acc = kf * len(terms)
            for n in range(n_chunks):
                last = n == n_chunks - 1
                # one PSUM tile spanning m_tiles banks; each matmul
                # writes within a single bank; one wide ACT copy drains.
                # The last chunk emits bank-major so each 512-wide half
                # completes early and its fold/DMA can chase it.
                pf = psf.tile([128, m_tiles * mw], mybir.dt.float32,
                              tag="pf", name="pf")
                for k in range(kf):
                    for ti, (lnm, rnm) in enumerate(terms):
                        for m in range(m_tiles):
                            acc = k * len(terms) + ti
                            nc.tensor.matmul(
                                pf[:, m * mw:(m + 1) * mw],
                                in_sb[lnm][k][:, n * 128:(n + 1) * 128],
                                in_sb[rnm][k][:, m * mw:(m + 1) * mw],
                                start=(acc == 0), stop=(acc == n_acc - 1))
                if n == n_chunks - 2:
                    # raw S rows to the host (ACT-issued DMA, off the
                    # critical path); fold reads PSUM directly
                    s_raw = schunk_pool.tile([128, m_shard],
                                             mybir.dt.float32, tag="schunk")
                    nc.scalar.copy(s_raw[:], pf[:])
                    nc.scalar.dma_start(s_tail[:], s_raw[:])
                    nc.vector.tensor_max(fold[:], fold[:], pf[:])
                elif last:
                    # fold first (it gates the fold DMAs, split across the
                    # SP and ACT queues); this chunk's row top-8 lands in
                    # the spare fv slot
                    nc.vector.tensor_max(fold[:], fold[:], pf[:])
                    nc.sync.dma_start(fold_out[:, :mw], fold[:, :mw])
                    nc.scalar.dma_start(fold_out[:, mw:], fold[:, mw:])
                    nc.vector.max(out=fv_sb[:, n * 8:(n + 1) * 8],
                                  in_=pf[:])
                    sl = slice(n * 8, (n + 1) * 8)
                    nc.sync.dma_start(fwd_val[:, sl], fv_sb[:, sl])
                else:
                    src = schunk_pool.tile([128, m_shard],
                                           mybir.dt.float32, tag="schunk")
                    nc.scalar.copy(src[:], pf[:])
                    # backward fold first: it is the tail's dependence chain
                    if n == 0:
                        nc.vector.tensor_copy(fold[:], src[:])
                    else:
                        nc.vector.tensor_max(fold[:], fold[:], src[:])
                    nc.vector.max(out=fv_sb[:, n * 8:(n + 1) * 8],
                                  in_=src[:])
                    # stream forward outputs out as they complete
                    if (n + 1) % (n_chunks // 4) == 0:
                        sl = slice((n + 1 - n_chunks // 4) * 8, (n + 1) * 8)
                        nc.sync.dma_start(fwd_val[:, sl], fv_sb[:, sl])
                    elif n == n_chunks - 3:
                        sl = slice((n_chunks - n_chunks // 4) * 8,
                                   (n + 1) * 8)
                        nc.sync.dma_start(fwd_val[:, sl], fv_sb[:, sl])

    nc.compile()
    return nc


_KERNEL_CACHE = {}


def get_kernel(dtype_mode=DTYPE_MODE):
    if dtype_mode not in _KERNEL_CACHE:
        _KERNEL_CACHE[dtype_mode] = build_kernel(dtype_mode=dtype_mode)
    return _KERNEL_CACHE[dtype_mode]


# --------------------------------------------------------------------------
# Host side
# --------------------------------------------------------------------------

def _decode_top8(arr, chunks):
    """[128, chunks*8] -> [chunks*128, 8] with row r = chunk*128 + partition."""
    return arr.reshape(128, chunks, 8).transpose(1, 0, 2).reshape(chunks * 128, 8)


def run_device(descriptors0, descriptors1, dtype_mode=DTYPE_MODE):
    """Run the SPMD kernel on 8 cores. Returns per-core raw outputs."""
    from concourse.bass_utils import run_bass_kernel_spmd

    nc = get_kernel(dtype_mode)
    d0 = np.ascontiguousarray(descriptors0[0]).astype(np.float32, copy=False)
    d1 = np.ascontiguousarray(descriptors1[0]).astype(np.float32, copy=False)
    kf = F_DIM // 128

    def shard(a, c):
        return np.ascontiguousarray(
            a[:, c * M_SHARD:(c + 1) * M_SHARD]).reshape(kf, 128, M_SHARD)

    if dtype_mode == "fp32":
        d0r = d0.reshape(kf, 128, N_KPTS)
        in_maps = [{"d0": d0r, "d1": shard(d1, c)} for c in range(N_CORES)]
    else:
        h0, h0s, l0s = _split_f16(d0)
        h1, h1s, l1s = _split_f16(d1)
        d0m = {"d0h": h0.reshape(kf, 128, N_KPTS),
               "d0hs": h0s.reshape(kf, 128, N_KPTS),
               "d0ls": l0s.reshape(kf, 128, N_KPTS)}
        in_maps = [dict(d0m, d1h=shard(h1, c), d1hs=shard(h1s, c),
                        d1ls=shard(l1s, c)) for c in range(N_CORES)]
    last_err = None
    for _attempt in range(3):
        try:
            res = run_bass_kernel_spmd(nc, in_maps, list(range(N_CORES)))
            # materialize device arrays now so transient device failures
            # surface inside this retry loop, not later in postprocess
            return [{k: np.asarray(v) for k, v in r.items()}
                    for r in res.results]
        except Exception as e:  # rare transient device-unrecoverable flakes
            last_err = e
    raise last_err


def postprocess(results):
    """Merge per-core device outputs into the reference's 4 output arrays."""
    n = N_KPTS
    n_chunks = n // 128

    # ---- forward: merge per-core top-2 into global top-2 ----
    m1 = np.empty((N_CORES, n), np.float32)
    m2 = np.empty((N_CORES, n), np.float32)
    for c in range(N_CORES):
        vals = _decode_top8(results[c]["fwd_val"], n_chunks)
        m1[c] = vals[:, 0]
        m2[c] = vals[:, 1]
        r62 = (n_chunks - 2) * 128
        r63 = (n_chunks - 1) * 128
        # chunk n-2 arrives as raw S rows; its top-2 is computed here
        s_tail = results[c]["s_tail"]              # [128, M_SHARD]
        top2 = -np.partition(-s_tail, 1, axis=1)[:, :2]
        m1[c, r62:r63] = top2[:, 0]
        m2[c, r62:r63] = top2[:, 1]

    w = np.argmax(m1, axis=0)                      # first max on ties
    rows = np.arange(n)
    s1 = m1[w, rows]
    m1_masked = m1.copy()
    m1_masked[w, rows] = -np.inf
    s2 = np.maximum(m1_masked.max(axis=0), m2[w, rows]).astype(np.float32)

    # ---- backward: reduce the per-lane fold over its 128 lanes ----
    cm1 = np.empty(M_KPTS, np.float32)
    cm2 = np.empty(M_KPTS, np.float32)
    lane = np.empty(M_KPTS, np.int64)
    for c in range(N_CORES):
        f = results[c]["fold"]                     # [128, M_SHARD]
        sl = slice(c * M_SHARD, (c + 1) * M_SHARD)
        lane[sl] = np.argmax(f, axis=0)
        cm1[sl] = f[lane[sl], np.arange(M_SHARD)]
        cm2[sl] = np.partition(f, 126, axis=0)[126]

    # ---- exact reference arithmetic (float32) ----
    def dist(s):
        return SQRT_2 * np.sqrt(np.maximum(ONE - s, CLIP_LO))

    fd1, fd2 = dist(s1), dist(s2)
    fwd_ok = (fd1 / fd2) < np.float32(1.0)
    bd1, bd2 = dist(cm1), dist(cm2)
    bck_ok = (bd1 / bd2) < np.float32(1.0)

    # ---- mutual NN + index recovery by exact value matching ----
    # Row i matches column j iff s1[i] == cm1[j] (bitwise: both sides are
    # the same f32 S element).  Search row i's winning core's 1024 column
    # maxes for the value s1[i].  A hit from a DIFFERENT dot product that
    # happens to collide bitwise is filtered by the argmax-lane check
    # (i % 128 must equal the column's winning lane); residual ambiguity
    # is reported (expected never).
    fwd_nn = np.zeros(n, np.int64)
    found = np.zeros(n, bool)
    ambig = 0
    for c in range(N_CORES):
        rows_c = np.nonzero(w == c)[0]
        if rows_c.size == 0:
            continue
        M1c = cm1[c * M_SHARD:(c + 1) * M_SHARD]
        order = np.argsort(M1c, kind="stable")
        sv = M1c[order]
        tgt = s1[rows_c]
        lo = np.searchsorted(sv, tgt, side="left")
        hi = np.searchsorted(sv, tgt, side="right")
        lane_c = lane[c * M_SHARD:(c + 1) * M_SHARD]
        for ri, l, h in zip(rows_c, lo, hi):
            if h == l:
                continue
            cands = [j for j in order[l:h] if lane_c[j] == ri % 128]
            if len(cands) == 1:
                fwd_nn[ri] = cands[0] + c * M_SHARD
                found[ri] = True
            elif len(cands) > 1:
                ambig += 1
    if ambig:
        print(f"WARNING: {ambig} ambiguous column-max value collisions")

    mutual = fwd_ok & found & bck_ok[fwd_nn]

    indices0 = np.where(mutual, fwd_nn, -1)[None, :].astype(np.int32)
    mscores0 = (indices0 > 0).astype(np.int32)
    matches1 = np.full((1, M_KPTS), -1, dtype=np.int32)
    mscores1 = np.zeros((1, M_KPTS), dtype=np.float32)
    return indices0, matches1, mscores0, mscores1


def kernel(descriptors0, descriptors1, keypoints0, keypoints1):
    results = run_device(descriptors0, descriptors1)
    return postprocess(results)


# revision 43
# speedup vs baseline: 1.0064x; 1.0064x over previous
"""Trainium2 Bass kernel for Disk descriptor mutual-NN matching (retrieval_knn).

Strategy (8 NeuronCores, shard descriptors1 columns M across cores):
  - Each core c holds full d0 [256, 8192] and its d1 shard [256, 1024].
  - S_c = d0.T @ d1_c via f16x3 split matmuls (3 f16 passes = 3/4 the fp32
    PE time; products exact below fp32 accumulation noise).
  - Forward: per-row top-8 values over the core's 1024 columns (DVE
    InstMax).  No index pass: indices are recovered on the host (below).
  - Backward: per-lane running elementwise max over the 64 row-chunks
    (DVE tensor_max fold, 1 op/chunk) -> fold[p, j] = max over rows
    {p + 128k} of column j.  The [128, 1024] fold state is DMAed out and
    the host reduces over the 128 lanes: exact column max M1[j]
    (bit-identical to the forward S values), its lane, and a cross-lane
    runner-up M2x[j] (== true second unless the column's top-2 share a
    lane, which only matters on exact value ties -- where the reference
    fails the ratio test anyway; with rt=1.0 the ratio test is a pure
    tie detector).
  - Host mutual test: row i is matched to column j iff its top value
    s1[i] IS the column max: s1[i] == M1[j], exact because both sides
    are the same f32 bits from the same PSUM drain.  This also recovers
    the matched column INDEX by value search (unique absent f32 bit
    collisions between distinct dot products, which are detected).
"""

import sys

if "/opt/trn_rl_repo" not in sys.path:
    sys.path.insert(0, "/opt/trn_rl_repo")

import numpy as np

N_KPTS = 8192
M_KPTS = 8192
F_DIM = 256
N_CORES = 8
M_SHARD = M_KPTS // N_CORES  # 1024

SQRT_2 = np.float32(1.414213)
CLIP_LO = np.float32(1e-6)
ONE = np.float32(1.0)

DTYPE_MODE = "f16x3"


def _split_f16(a32):
    """f32 -> (h, h/32, 32*(a-h)) as float16, with f16-subnormal highs
    flushed into the residual so no information rides on f16 subnormals."""
    h = a32.astype(np.float16)
    h[np.abs(a32) < 6.104e-5] = np.float16(0)
    l = a32 - h.astype(np.float32)
    h_s = (h.astype(np.float32) / 32.0).astype(np.float16)
    l_s = (l * 32.0).astype(np.float16)
    return h, h_s, l_s


# --------------------------------------------------------------------------
# Device kernel builder
# --------------------------------------------------------------------------

def build_kernel(n_rows=N_KPTS, m_shard=M_SHARD, f_dim=F_DIM,
                 dtype_mode=DTYPE_MODE):
    """Build the per-core SPMD Bass program.

    Inputs (per core, f16 split terms):
      d0h/d0hs/d0ls: [kf, 128, n_rows]   (descriptors0, K-chunked)
      d1h/d1hs/d1ls: [kf, 128, m_shard]  (this core's descriptors1 shard)
    Outputs (per core):
      fwd_val [128, n_chunks*8] f32  (row top-8 per 128-row chunk)
      fold    [128, m_shard] f32     (per-lane column maxes)
    """
    import concourse.bacc as bacc
    import concourse.mybir as mybir
    import concourse.tile as tile

    kf = f_dim // 128
    n_chunks = n_rows // 128          # row chunks
    m_tiles = max(1, m_shard // 512)  # 512-wide column tiles (PSUM banks)
    mw = min(512, m_shard)
    assert m_shard % 128 == 0 and f_dim % 128 == 0

    nc = bacc.Bacc("TRN2", target_bir_lowering=False, debug=False,
                   num_devices=1)

    if dtype_mode == "fp32":
        in_names = ["d0", "d1"]
        in_dt = mybir.dt.float32
    else:
        in_names = ["d0h", "d0hs", "d0ls", "d1h", "d1hs", "d1ls"]
        in_dt = mybir.dt.float16
    in_dram = {}
    for nm in in_names:
        nw = n_rows if nm.startswith("d0") else m_shard
        in_dram[nm] = nc.dram_tensor(nm, [kf, 128, nw], in_dt,
                                     kind="ExternalInput")
    fwd_val = nc.dram_tensor("fwd_val", [128, n_chunks * 8], mybir.dt.float32,
                             kind="ExternalOutput")
    fold_out = nc.dram_tensor("fold", [128, m_shard], mybir.dt.float32,
                              kind="ExternalOutput")
    # raw S rows of the last two chunks; the host does their forward
    # top-8 AND their backward contribution, so no device work remains
    # after the last matmuls except two small copies + DMAs
    s_tail = nc.dram_tensor("s_tail", [2, 128, m_shard], mybir.dt.float32,
                            kind="ExternalOutput")

    with tile.TileContext(nc) as tc:
        with tc.tile_pool(name="persist", bufs=1) as persist, \
             tc.tile_pool(name="schunk", bufs=4) as schunk_pool, \
             tc.tile_pool(name="outs", bufs=1) as outs_pool, \
             tc.tile_pool(name="psf", bufs=3, space="PSUM") as psf:

            # resident inputs; d0 loads split along n so early fwd units
            # unblock before the full load completes
            in_sb = {}
            for nm in in_names:
                nw = n_rows if nm.startswith("d0") else m_shard
                in_sb[nm] = [persist.tile([128, nw], in_dt,
                                          name=f"{nm}sb{k}", tag=f"{nm}sb{k}")
                             for k in range(kf)]
            # input loads: the 12 tiles chunk 0 consumes are co-scheduled
            # by first-use time across the SP / ACT / Pool sequencers so
            # none of them serializes behind SP's 565 ns issue slot;
            # the remaining d0 row-pieces stream on SP well ahead of use
            n_split = 8 if n_rows % 1024 == 0 else 1
            d0n = [nm for nm in in_names if nm.startswith("d0")]
            p0 = slice(0, n_rows // n_split)
            if dtype_mode == "fp32":
                for k in range(kf):
                    nc.sync.dma_start(in_sb["d0"][k][:, p0],
                                      in_dram["d0"][k][:, p0])
                    nc.scalar.dma_start(in_sb["d1"][k][:], in_dram["d1"][k])
            else:
                for k in range(kf):
                    nc.sync.dma_start(in_sb["d0h"][k][:, p0],
                                      in_dram["d0h"][k][:, p0])
                    nc.sync.dma_start(in_sb["d0hs"][k][:, p0],
                                      in_dram["d0hs"][k][:, p0])
                for k in range(kf):
                    nc.scalar.dma_start(in_sb["d1h"][k][:], in_dram["d1h"][k])
                    nc.scalar.dma_start(in_sb["d1ls"][k][:],
                                        in_dram["d1ls"][k])
                    nc.scalar.dma_start(in_sb["d0ls"][k][:, p0],
                                        in_dram["d0ls"][k][:, p0])
                for k in range(kf):
                    nc.gpsimd.dma_start(in_sb["d1hs"][k][:],
                                        in_dram["d1hs"][k])
            for p in range(1, n_split):
                sl = slice(p * n_rows // n_split, (p + 1) * n_rows // n_split)
                for k in range(kf):
                    for nm in d0n:
                        nc.sync.dma_start(in_sb[nm][k][:, sl],
                                          in_dram[nm][k][:, sl])
            if dtype_mode == "fp32":
                terms = [("d0", "d1")]
            else:
                terms = [("d0h", "d1h"), ("d0hs", "d1ls"), ("d0ls", "d1hs")]

            zeros = persist.tile([128, 128], mybir.dt.float32, name="zeros")
            nc.vector.memset(zeros[:], 0)
            # warm-up matmul: starts the PE p-state ramp clock while input
            # DMAs are still streaming (zeros need no DMA)
            warm = psf.tile([128, 8], mybir.dt.float32, tag="pf", name="warm",
                            padded_shape=[128, m_tiles * mw])
            nc.tensor.matmul(warm[:], zeros[:], zeros[:, :8],
                             start=True, stop=True)

            fv_sb = outs_pool.tile([128, n_chunks * 8], mybir.dt.float32)
            # per-lane running column max over row chunks
            fold = outs_pool.tile([128, m_shard], mybir.dt.float32,
                                  name="fold")

            n_acc = kf * len(terms)
            for n in range(n_chunks):
                last = n == n_chunks - 1
                # one PSUM tile spanning m_tiles banks; each matmul
                # writes within a single bank; one wide ACT copy drains.
                # The last chunk emits bank-major so each 512-wide half
                # completes early and its fold/DMA can chase it.
                # the last chunk emits bank-major into SEPARATE PSUM tiles
                # (Tile tracks deps per tile) so each 512-wide bank's drain
                # copy can chase its bank without waiting for the other
                if last:
                    banks = [psf.tile([128, mw], mybir.dt.float32,
                                      tag="pf", name=f"pfl{m}",
                                      padded_shape=[128, m_tiles * mw])
                             for m in range(m_tiles)]
                    bank_ap = [banks[m][:, :mw] for m in range(m_tiles)]
                    m_order = [(k, ti, m) for m in range(m_tiles)
                               for k in range(kf)
                               for ti in range(len(terms))]
                else:
                    pf = psf.tile([128, m_tiles * mw], mybir.dt.float32,
                                  tag="pf", name="pf")
                    bank_ap = [pf[:, m * mw:(m + 1) * mw]
                               for m in range(m_tiles)]
                    m_order = [(k, ti, m) for k in range(kf)
                               for ti in range(len(terms))
                               for m in range(m_tiles)]
                for k, ti, m in m_order:
                    lnm, rnm = terms[ti]
                    acc = k * len(terms) + ti
                    nc.tensor.matmul(
                        bank_ap[m],
                        in_sb[lnm][k][:, n * 128:(n + 1) * 128],
                        in_sb[rnm][k][:, m * mw:(m + 1) * mw],
                        start=(acc == 0), stop=(acc == n_acc - 1))
                if n == n_chunks - 2:
                    # raw S rows out via the gpsimd (SWDGE) queue, which
                    # does not contend with the SP/ACT tail DMAs
                    s_raw = schunk_pool.tile([128, m_shard],
                                             mybir.dt.float32, tag="schunk")
                    nc.scalar.copy(s_raw[:], pf[:])
                    for q in range(2):
                        sl = slice(q * mw, (q + 1) * mw)
                        nc.gpsimd.dma_start(s_tail[0][:, sl], s_raw[:, sl])
                elif last:
                    # per-bank drain copies chase the bank-major matmuls;
                    # halves leave on the ACT and SP queues in parallel
                    s_raw = schunk_pool.tile([128, m_shard],
                                             mybir.dt.float32, tag="schunk")
                    nc.scalar.copy(s_raw[:, :mw], bank_ap[0])
                    nc.scalar.dma_start(s_tail[1][:, :mw], s_raw[:, :mw])
                    nc.scalar.copy(s_raw[:, mw:], bank_ap[-1])
                    nc.sync.dma_start(s_tail[1][:, mw:], s_raw[:, mw:])
                else:
                    src = schunk_pool.tile([128, m_shard],
                                           mybir.dt.float32, tag="schunk")
                    nc.scalar.copy(src[:], pf[:])
                    # backward fold first: it is the tail's dependence chain
                    if n == 0:
                        nc.vector.tensor_copy(fold[:], src[:])
                    else:
                        nc.vector.tensor_max(fold[:], fold[:], src[:])
                    nc.vector.max(out=fv_sb[:, n * 8:(n + 1) * 8],
                                  in_=src[:])
                    if n == n_chunks - 3:
                        # the fold is complete: ship it while the last
                        # chunk's matmuls still run
                        nc.sync.dma_start(fold_out[:, :mw], fold[:, :mw])
                        nc.sync.dma_start(fold_out[:, mw:], fold[:, mw:])
                        sl = slice((n_chunks - n_chunks // 4) * 8,
                                   (n + 1) * 8)
                        nc.scalar.dma_start(fwd_val[:, sl], fv_sb[:, sl])
                    elif (n + 1) % (n_chunks // 4) == 0:
                        # stream forward outputs out as they complete
                        sl = slice((n + 1 - n_chunks // 4) * 8, (n + 1) * 8)
                        nc.sync.dma_start(fwd_val[:, sl], fv_sb[:, sl])

    nc.compile()
    return nc


_KERNEL_CACHE = {}


def get_kernel(dtype_mode=DTYPE_MODE):
    if dtype_mode not in _KERNEL_CACHE:
        _KERNEL_CACHE[dtype_mode] = build_kernel(dtype_mode=dtype_mode)
    return _KERNEL_CACHE[dtype_mode]


# --------------------------------------------------------------------------
# Host side
# --------------------------------------------------------------------------

def _decode_top8(arr, chunks):
    """[128, chunks*8] -> [chunks*128, 8] with row r = chunk*128 + partition."""
    return arr.reshape(128, chunks, 8).transpose(1, 0, 2).reshape(chunks * 128, 8)


def run_device(descriptors0, descriptors1, dtype_mode=DTYPE_MODE):
    """Run the SPMD kernel on 8 cores. Returns per-core raw outputs."""
    from concourse.bass_utils import run_bass_kernel_spmd

    nc = get_kernel(dtype_mode)
    d0 = np.ascontiguousarray(descriptors0[0]).astype(np.float32, copy=False)
    d1 = np.ascontiguousarray(descriptors1[0]).astype(np.float32, copy=False)
    kf = F_DIM // 128

    def shard(a, c):
        return np.ascontiguousarray(
            a[:, c * M_SHARD:(c + 1) * M_SHARD]).reshape(kf, 128, M_SHARD)

    if dtype_mode == "fp32":
        d0r = d0.reshape(kf, 128, N_KPTS)
        in_maps = [{"d0": d0r, "d1": shard(d1, c)} for c in range(N_CORES)]
    else:
        h0, h0s, l0s = _split_f16(d0)
        h1, h1s, l1s = _split_f16(d1)
        d0m = {"d0h": h0.reshape(kf, 128, N_KPTS),
               "d0hs": h0s.reshape(kf, 128, N_KPTS),
               "d0ls": l0s.reshape(kf, 128, N_KPTS)}
        in_maps = [dict(d0m, d1h=shard(h1, c), d1hs=shard(h1s, c),
                        d1ls=shard(l1s, c)) for c in range(N_CORES)]
    last_err = None
    for _attempt in range(3):
        try:
            res = run_bass_kernel_spmd(nc, in_maps, list(range(N_CORES)))
            # materialize device arrays now so transient device failures
            # surface inside this retry loop, not later in postprocess
            return [{k: np.asarray(v) for k, v in r.items()}
                    for r in res.results]
        except Exception as e:  # rare transient device-unrecoverable flakes
            last_err = e
    raise last_err


def postprocess(results):
    """Merge per-core device outputs into the reference's 4 output arrays."""
    n = N_KPTS
    n_chunks = n // 128

    # ---- forward: merge per-core top-2 into global top-2 ----
    m1 = np.empty((N_CORES, n), np.float32)
    m2 = np.empty((N_CORES, n), np.float32)
    for c in range(N_CORES):
        vals = _decode_top8(results[c]["fwd_val"], n_chunks)
        m1[c] = vals[:, 0]
        m2[c] = vals[:, 1]
        r62 = (n_chunks - 2) * 128
        # the last two chunks arrive as raw S rows; top-2 computed here
        s_tail = results[c]["s_tail"]              # [2, 128, M_SHARD]
        top2 = -np.partition(-s_tail, 1, axis=2)[:, :, :2]
        m1[c, r62:] = top2[:, :, 0].reshape(-1)
        m2[c, r62:] = top2[:, :, 1].reshape(-1)

    w = np.argmax(m1, axis=0)                      # first max on ties
    rows = np.arange(n)
    s1 = m1[w, rows]
    m1_masked = m1.copy()
    m1_masked[w, rows] = -np.inf
    s2 = np.maximum(m1_masked.max(axis=0), m2[w, rows]).astype(np.float32)

    # ---- backward: reduce fold lanes (chunks 0..n-3) + raw tail rows ----
    cm1 = np.empty(M_KPTS, np.float32)
    cm2 = np.empty(M_KPTS, np.float32)
    lane = np.empty(M_KPTS, np.int64)       # winner lane (i % 128)
    erow = np.empty(M_KPTS, np.int64)       # exact winner row, -1 unknown
    for c in range(N_CORES):
        f = results[c]["fold"]                     # [128, M_SHARD]
        s_tail = results[c]["s_tail"]              # [2, 128, M_SHARD]
        stack = np.concatenate([f, s_tail[0], s_tail[1]], axis=0)  # [384, m]
        sl = slice(c * M_SHARD, (c + 1) * M_SHARD)
        win = np.argmax(stack, axis=0)
        cm1[sl] = stack[win, np.arange(M_SHARD)]
        cm2[sl] = np.partition(stack, 382, axis=0)[382]
        lane[sl] = win % 128
        # winners from the raw tail chunks pin the exact row
        erow[sl] = np.where(win >= 128,
                            (n_chunks - 2 + win // 128 - 1) * 128
                            + win % 128, -1)

    # ---- exact reference arithmetic (float32) ----
    def dist(s):
        return SQRT_2 * np.sqrt(np.maximum(ONE - s, CLIP_LO))

    fd1, fd2 = dist(s1), dist(s2)
    fwd_ok = (fd1 / fd2) < np.float32(1.0)
    bd1, bd2 = dist(cm1), dist(cm2)
    bck_ok = (bd1 / bd2) < np.float32(1.0)

    # ---- mutual NN + index recovery by exact value matching ----
    # Row i matches column j iff s1[i] == cm1[j] (bitwise: both sides are
    # the same f32 S element).  Search row i's winning core's 1024 column
    # maxes for the value s1[i].  A hit from a DIFFERENT dot product that
    # happens to collide bitwise is filtered by the argmax-lane check
    # (i % 128 must equal the column's winning lane); residual ambiguity
    # is reported (expected never).
    fwd_nn = np.zeros(n, np.int64)
    found = np.zeros(n, bool)
    ambig = 0
    for c in range(N_CORES):
        rows_c = np.nonzero(w == c)[0]
        if rows_c.size == 0:
            continue
        M1c = cm1[c * M_SHARD:(c + 1) * M_SHARD]
        order = np.argsort(M1c, kind="stable")
        sv = M1c[order]
        tgt = s1[rows_c]
        lo = np.searchsorted(sv, tgt, side="left")
        hi = np.searchsorted(sv, tgt, side="right")
        lane_c = lane[c * M_SHARD:(c + 1) * M_SHARD]
        erow_c = erow[c * M_SHARD:(c + 1) * M_SHARD]
        r_tail = (N_KPTS // 128 - 2) * 128
        for ri, l, h in zip(rows_c, lo, hi):
            if h == l:
                continue
            cands = [j for j in order[l:h]
                     if (ri == erow_c[j] if erow_c[j] >= 0
                         else (lane_c[j] == ri % 128 and ri < r_tail))]
            if len(cands) == 1:
                fwd_nn[ri] = cands[0] + c * M_SHARD
                found[ri] = True
            elif len(cands) > 1:
                ambig += 1
    if ambig:
        print(f"WARNING: {ambig} ambiguous column-max value collisions")

    mutual = fwd_ok & found & bck_ok[fwd_nn]

    indices0 = np.where(mutual, fwd_nn, -1)[None, :].astype(np.int32)
    mscores0 = (indices0 > 0).astype(np.int32)
    matches1 = np.full((1, M_KPTS), -1, dtype=np.int32)
    mscores1 = np.zeros((1, M_KPTS), dtype=np.float32)
    return indices0, matches1, mscores0, mscores1


def kernel(descriptors0, descriptors1, keypoints0, keypoints1):
    results = run_device(descriptors0, descriptors1)
    return postprocess(results)


# revision 46
# speedup vs baseline: 1.0121x; 1.0057x over previous
"""Trainium2 Bass kernel for Disk descriptor mutual-NN matching (retrieval_knn).

Strategy (8 NeuronCores, shard descriptors1 columns M across cores):
  - Each core c holds full d0 [256, 8192] and its d1 shard [256, 1024].
  - S_c = d0.T @ d1_c via f16x3 split matmuls (3 f16 passes = 3/4 the fp32
    PE time; products exact below fp32 accumulation noise).
  - Forward: per-row top-8 values over the core's 1024 columns (DVE
    InstMax).  No index pass: indices are recovered on the host (below).
  - Backward: per-lane running elementwise max over the 64 row-chunks
    (DVE tensor_max fold, 1 op/chunk) -> fold[p, j] = max over rows
    {p + 128k} of column j.  The [128, 1024] fold state is DMAed out and
    the host reduces over the 128 lanes: exact column max M1[j]
    (bit-identical to the forward S values), its lane, and a cross-lane
    runner-up M2x[j] (== true second unless the column's top-2 share a
    lane, which only matters on exact value ties -- where the reference
    fails the ratio test anyway; with rt=1.0 the ratio test is a pure
    tie detector).
  - Host mutual test: row i is matched to column j iff its top value
    s1[i] IS the column max: s1[i] == M1[j], exact because both sides
    are the same f32 bits from the same PSUM drain.  This also recovers
    the matched column INDEX by value search (unique absent f32 bit
    collisions between distinct dot products, which are detected).
"""

import sys

if "/opt/trn_rl_repo" not in sys.path:
    sys.path.insert(0, "/opt/trn_rl_repo")

import numpy as np

N_KPTS = 8192
M_KPTS = 8192
F_DIM = 256
N_CORES = 8
M_SHARD = M_KPTS // N_CORES  # 1024

SQRT_2 = np.float32(1.414213)
CLIP_LO = np.float32(1e-6)
ONE = np.float32(1.0)

DTYPE_MODE = "f16x3"


def _split_f16(a32):
    """f32 -> (h, h/32, 32*(a-h)) as float16, with f16-subnormal highs
    flushed into the residual so no information rides on f16 subnormals."""
    h = a32.astype(np.float16)
    h[np.abs(a32) < 6.104e-5] = np.float16(0)
    l = a32 - h.astype(np.float32)
    h_s = (h.astype(np.float32) / 32.0).astype(np.float16)
    l_s = (l * 32.0).astype(np.float16)
    return h, h_s, l_s


# --------------------------------------------------------------------------
# Device kernel builder
# --------------------------------------------------------------------------

def build_kernel(n_rows=N_KPTS, m_shard=M_SHARD, f_dim=F_DIM,
                 dtype_mode=DTYPE_MODE):
    """Build the per-core SPMD Bass program.

    Inputs (per core, f16 split terms):
      d0h/d0hs/d0ls: [kf, 128, n_rows]   (descriptors0, K-chunked)
      d1h/d1hs/d1ls: [kf, 128, m_shard]  (this core's descriptors1 shard)
    Outputs (per core):
      fwd_val [128, n_chunks*8] f32  (row top-8 per 128-row chunk)
      fold    [128, m_shard] f32     (per-lane column maxes)
    """
    import concourse.bacc as bacc
    import concourse.mybir as mybir
    import concourse.tile as tile

    kf = f_dim // 128
    n_chunks = n_rows // 128          # row chunks
    m_tiles = max(1, m_shard // 512)  # 512-wide column tiles (PSUM banks)
    mw = min(512, m_shard)
    assert m_shard % 128 == 0 and f_dim % 128 == 0

    nc = bacc.Bacc("TRN2", target_bir_lowering=False, debug=False,
                   num_devices=1)

    if dtype_mode == "fp32":
        in_names = ["d0", "d1"]
        in_dt = mybir.dt.float32
    else:
        in_names = ["d0h", "d0hs", "d0ls", "d1h", "d1hs", "d1ls"]
        in_dt = mybir.dt.float16
    in_dram = {}
    for nm in in_names:
        nw = n_rows if nm.startswith("d0") else m_shard
        in_dram[nm] = nc.dram_tensor(nm, [kf, 128, nw], in_dt,
                                     kind="ExternalInput")
    fwd_val = nc.dram_tensor("fwd_val", [128, n_chunks * 8], mybir.dt.float32,
                             kind="ExternalOutput")
    fold_out = nc.dram_tensor("fold", [128, m_shard], mybir.dt.float32,
                              kind="ExternalOutput")
    # raw S rows of the last two chunks; the host does their forward
    # top-8 AND their backward contribution, so no device work remains
    # after the last matmuls except two small copies + DMAs
    s_tail = nc.dram_tensor("s_tail", [2, 128, m_shard], mybir.dt.float32,
                            kind="ExternalOutput")

    with tile.TileContext(nc) as tc:
        with tc.tile_pool(name="persist", bufs=1) as persist, \
             tc.tile_pool(name="schunk", bufs=4) as schunk_pool, \
             tc.tile_pool(name="outs", bufs=1) as outs_pool, \
             tc.tile_pool(name="psf", bufs=3, space="PSUM") as psf:

            # resident inputs; d0 loads split along n so early fwd units
            # unblock before the full load completes
            in_sb = {}
            for nm in in_names:
                nw = n_rows if nm.startswith("d0") else m_shard
                in_sb[nm] = [persist.tile([128, nw], in_dt,
                                          name=f"{nm}sb{k}", tag=f"{nm}sb{k}")
                             for k in range(kf)]
            # input loads: the 12 tiles chunk 0 consumes are co-scheduled
            # by first-use time across the SP / ACT / Pool sequencers so
            # none of them serializes behind SP's 565 ns issue slot;
            # the remaining d0 row-pieces stream on SP well ahead of use
            n_split = 8 if n_rows % 1024 == 0 else 1
            d0n = [nm for nm in in_names if nm.startswith("d0")]
            p0 = slice(0, n_rows // n_split)
            if dtype_mode == "fp32":
                for k in range(kf):
                    nc.sync.dma_start(in_sb["d0"][k][:, p0],
                                      in_dram["d0"][k][:, p0])
                    nc.scalar.dma_start(in_sb["d1"][k][:], in_dram["d1"][k])
            else:
                # the very first matmul's two operands load smallest+first:
                # a 128-row d0h piece on SP and the first d1h half via the
                # low-latency SWDGE queue
                tiny = slice(0, 128)
                nc.sync.dma_start(in_sb["d0h"][0][:, tiny],
                                  in_dram["d0h"][0][:, tiny])
                nc.gpsimd.dma_start(in_sb["d1h"][0][:, :mw],
                                    in_dram["d1h"][0][:, :mw])
                rest = slice(128, n_rows // n_split)
                nc.sync.dma_start(in_sb["d0h"][0][:, rest],
                                  in_dram["d0h"][0][:, rest])
                for k in range(kf):
                    if k > 0:
                        nc.sync.dma_start(in_sb["d0h"][k][:, p0],
                                          in_dram["d0h"][k][:, p0])
                    nc.sync.dma_start(in_sb["d0hs"][k][:, p0],
                                      in_dram["d0hs"][k][:, p0])
                nc.scalar.dma_start(in_sb["d1h"][0][:, mw:],
                                    in_dram["d1h"][0][:, mw:])
                for k in range(kf):
                    if k > 0:
                        nc.scalar.dma_start(in_sb["d1h"][k][:],
                                            in_dram["d1h"][k])
                    nc.scalar.dma_start(in_sb["d1ls"][k][:],
                                        in_dram["d1ls"][k])
                    nc.scalar.dma_start(in_sb["d0ls"][k][:, p0],
                                        in_dram["d0ls"][k][:, p0])
                for k in range(kf):
                    nc.gpsimd.dma_start(in_sb["d1hs"][k][:],
                                        in_dram["d1hs"][k])
            for p in range(1, n_split):
                sl = slice(p * n_rows // n_split, (p + 1) * n_rows // n_split)
                for k in range(kf):
                    for nm in d0n:
                        nc.sync.dma_start(in_sb[nm][k][:, sl],
                                          in_dram[nm][k][:, sl])
            if dtype_mode == "fp32":
                terms = [("d0", "d1")]
            else:
                terms = [("d0h", "d1h"), ("d0hs", "d1ls"), ("d0ls", "d1hs")]

            zeros = persist.tile([128, 128], mybir.dt.float32, name="zeros")
            nc.vector.memset(zeros[:], 0)
            # warm-up matmul: starts the PE p-state ramp clock while input
            # DMAs are still streaming (zeros need no DMA)
            warm = psf.tile([128, 8], mybir.dt.float32, tag="pf", name="warm",
                            padded_shape=[128, m_tiles * mw])
            nc.tensor.matmul(warm[:], zeros[:], zeros[:, :8],
                             start=True, stop=True)

            fv_sb = outs_pool.tile([128, n_chunks * 8], mybir.dt.float32)
            # per-lane running column max over row chunks
            fold = outs_pool.tile([128, m_shard], mybir.dt.float32,
                                  name="fold")

            n_acc = kf * len(terms)
            for n in range(n_chunks):
                last = n == n_chunks - 1
                # one PSUM tile spanning m_tiles banks; each matmul
                # writes within a single bank; one wide ACT copy drains.
                # The last chunk emits bank-major so each 512-wide half
                # completes early and its fold/DMA can chase it.
                # the last chunk emits segment-major into SEPARATE PSUM
                # tiles (Tile tracks deps per tile) with shrinking widths,
                # so each segment's drain copy+DMA chases its completion
                if last:
                    if m_tiles == 2:
                        segs = [(0, mw), (mw, mw + mw // 2),
                                (mw + mw // 2, 2 * mw)]
                    else:
                        segs = [(m * mw, (m + 1) * mw)
                                for m in range(m_tiles)]
                    tiles = [psf.tile([128, b - a], mybir.dt.float32,
                                      tag="pf", name=f"pfl{si}",
                                      padded_shape=[128, m_tiles * mw])
                             for si, (a, b) in enumerate(segs)]
                    seg_ap = [t[:, :] for t in tiles]
                    m_order = [(k, ti, si) for si in range(len(segs))
                               for k in range(kf)
                               for ti in range(len(terms))]
                else:
                    segs = [(m * mw, (m + 1) * mw) for m in range(m_tiles)]
                    pf = psf.tile([128, m_tiles * mw], mybir.dt.float32,
                                  tag="pf", name="pf")
                    seg_ap = [pf[:, a:b] for a, b in segs]
                    m_order = [(k, ti, m) for k in range(kf)
                               for ti in range(len(terms))
                               for m in range(m_tiles)]
                for k, ti, m in m_order:
                    lnm, rnm = terms[ti]
                    acc = k * len(terms) + ti
                    nc.tensor.matmul(
                        seg_ap[m],
                        in_sb[lnm][k][:, n * 128:(n + 1) * 128],
                        in_sb[rnm][k][:, segs[m][0]:segs[m][1]],
                        start=(acc == 0), stop=(acc == n_acc - 1))
                if n == n_chunks - 2:
                    # raw S rows out via the gpsimd (SWDGE) queue, which
                    # does not contend with the SP/ACT tail DMAs
                    s_raw = schunk_pool.tile([128, m_shard],
                                             mybir.dt.float32, tag="schunk")
                    nc.scalar.copy(s_raw[:], pf[:])
                    for q in range(2):
                        sl = slice(q * mw, (q + 1) * mw)
                        nc.gpsimd.dma_start(s_tail[0][:, sl], s_raw[:, sl])
                elif last:
                    # per-segment drain copies chase the matmul segments;
                    # the first (widest) segment drains via ACT, later
                    # ones via the idle DVE, DMAs spread over ACT and SP
                    s_raw = schunk_pool.tile([128, m_shard],
                                             mybir.dt.float32, tag="schunk")
                    for si, (a, b) in enumerate(segs):
                        if si == 0:
                            nc.scalar.copy(s_raw[:, a:b], seg_ap[si])
                            nc.scalar.dma_start(s_tail[1][:, a:b],
                                                s_raw[:, a:b])
                        else:
                            nc.vector.tensor_copy(s_raw[:, a:b], seg_ap[si])
                            nc.sync.dma_start(s_tail[1][:, a:b],
                                              s_raw[:, a:b])
                else:
                    src = schunk_pool.tile([128, m_shard],
                                           mybir.dt.float32, tag="schunk")
                    nc.scalar.copy(src[:], pf[:])
                    # backward fold first: it is the tail's dependence chain
                    if n == 0:
                        nc.vector.tensor_copy(fold[:], src[:])
                    else:
                        nc.vector.tensor_max(fold[:], fold[:], src[:])
                    nc.vector.max(out=fv_sb[:, n * 8:(n + 1) * 8],
                                  in_=src[:])
                    if n == n_chunks - 3:
                        # the fold is complete: ship it while the last
                        # chunk's matmuls still run
                        nc.sync.dma_start(fold_out[:, :mw], fold[:, :mw])
                        nc.sync.dma_start(fold_out[:, mw:], fold[:, mw:])
                        sl = slice((n_chunks - n_chunks // 4) * 8,
                                   (n + 1) * 8)
                        nc.scalar.dma_start(fwd_val[:, sl], fv_sb[:, sl])
                    elif (n + 1) % (n_chunks // 4) == 0:
                        # stream forward outputs out as they complete
                        sl = slice((n + 1 - n_chunks // 4) * 8, (n + 1) * 8)
                        nc.sync.dma_start(fwd_val[:, sl], fv_sb[:, sl])

    nc.compile()
    return nc


_KERNEL_CACHE = {}


def get_kernel(dtype_mode=DTYPE_MODE):
    if dtype_mode not in _KERNEL_CACHE:
        _KERNEL_CACHE[dtype_mode] = build_kernel(dtype_mode=dtype_mode)
    return _KERNEL_CACHE[dtype_mode]


# --------------------------------------------------------------------------
# Host side
# --------------------------------------------------------------------------

def _decode_top8(arr, chunks):
    """[128, chunks*8] -> [chunks*128, 8] with row r = chunk*128 + partition."""
    return arr.reshape(128, chunks, 8).transpose(1, 0, 2).reshape(chunks * 128, 8)


def run_device(descriptors0, descriptors1, dtype_mode=DTYPE_MODE):
    """Run the SPMD kernel on 8 cores. Returns per-core raw outputs."""
    from concourse.bass_utils import run_bass_kernel_spmd

    nc = get_kernel(dtype_mode)
    d0 = np.ascontiguousarray(descriptors0[0]).astype(np.float32, copy=False)
    d1 = np.ascontiguousarray(descriptors1[0]).astype(np.float32, copy=False)
    kf = F_DIM // 128

    def shard(a, c):
        return np.ascontiguousarray(
            a[:, c * M_SHARD:(c + 1) * M_SHARD]).reshape(kf, 128, M_SHARD)

    if dtype_mode == "fp32":
        d0r = d0.reshape(kf, 128, N_KPTS)
        in_maps = [{"d0": d0r, "d1": shard(d1, c)} for c in range(N_CORES)]
    else:
        h0, h0s, l0s = _split_f16(d0)
        h1, h1s, l1s = _split_f16(d1)
        d0m = {"d0h": h0.reshape(kf, 128, N_KPTS),
               "d0hs": h0s.reshape(kf, 128, N_KPTS),
               "d0ls": l0s.reshape(kf, 128, N_KPTS)}
        in_maps = [dict(d0m, d1h=shard(h1, c), d1hs=shard(h1s, c),
                        d1ls=shard(l1s, c)) for c in range(N_CORES)]
    last_err = None
    for _attempt in range(3):
        try:
            res = run_bass_kernel_spmd(nc, in_maps, list(range(N_CORES)))
            # materialize device arrays now so transient device failures
            # surface inside this retry loop, not later in postprocess
            return [{k: np.asarray(v) for k, v in r.items()}
                    for r in res.results]
        except Exception as e:  # rare transient device-unrecoverable flakes
            last_err = e
    raise last_err


def postprocess(results):
    """Merge per-core device outputs into the reference's 4 output arrays."""
    n = N_KPTS
    n_chunks = n // 128

    # ---- forward: merge per-core top-2 into global top-2 ----
    m1 = np.empty((N_CORES, n), np.float32)
    m2 = np.empty((N_CORES, n), np.float32)
    for c in range(N_CORES):
        vals = _decode_top8(results[c]["fwd_val"], n_chunks)
        m1[c] = vals[:, 0]
        m2[c] = vals[:, 1]
        r62 = (n_chunks - 2) * 128
        # the last two chunks arrive as raw S rows; top-2 computed here
        s_tail = results[c]["s_tail"]              # [2, 128, M_SHARD]
        top2 = -np.partition(-s_tail, 1, axis=2)[:, :, :2]
        m1[c, r62:] = top2[:, :, 0].reshape(-1)
        m2[c, r62:] = top2[:, :, 1].reshape(-1)

    w = np.argmax(m1, axis=0)                      # first max on ties
    rows = np.arange(n)
    s1 = m1[w, rows]
    m1_masked = m1.copy()
    m1_masked[w, rows] = -np.inf
    s2 = np.maximum(m1_masked.max(axis=0), m2[w, rows]).astype(np.float32)

    # ---- backward: reduce fold lanes (chunks 0..n-3) + raw tail rows ----
    cm1 = np.empty(M_KPTS, np.float32)
    cm2 = np.empty(M_KPTS, np.float32)
    lane = np.empty(M_KPTS, np.int64)       # winner lane (i % 128)
    erow = np.empty(M_KPTS, np.int64)       # exact winner row, -1 unknown
    for c in range(N_CORES):
        f = results[c]["fold"]                     # [128, M_SHARD]
        s_tail = results[c]["s_tail"]              # [2, 128, M_SHARD]
        stack = np.concatenate([f, s_tail[0], s_tail[1]], axis=0)  # [384, m]
        sl = slice(c * M_SHARD, (c + 1) * M_SHARD)
        win = np.argmax(stack, axis=0)
        cm1[sl] = stack[win, np.arange(M_SHARD)]
        cm2[sl] = np.partition(stack, 382, axis=0)[382]
        lane[sl] = win % 128
        # winners from the raw tail chunks pin the exact row
        erow[sl] = np.where(win >= 128,
                            (n_chunks - 2 + win // 128 - 1) * 128
                            + win % 128, -1)

    # ---- exact reference arithmetic (float32) ----
    def dist(s):
        return SQRT_2 * np.sqrt(np.maximum(ONE - s, CLIP_LO))

    fd1, fd2 = dist(s1), dist(s2)
    fwd_ok = (fd1 / fd2) < np.float32(1.0)
    bd1, bd2 = dist(cm1), dist(cm2)
    bck_ok = (bd1 / bd2) < np.float32(1.0)

    # ---- mutual NN + index recovery by exact value matching ----
    # Row i matches column j iff s1[i] == cm1[j] (bitwise: both sides are
    # the same f32 S element).  Search row i's winning core's 1024 column
    # maxes for the value s1[i].  A hit from a DIFFERENT dot product that
    # happens to collide bitwise is filtered by the argmax-lane check
    # (i % 128 must equal the column's winning lane); residual ambiguity
    # is reported (expected never).
    fwd_nn = np.zeros(n, np.int64)
    found = np.zeros(n, bool)
    ambig = 0
    for c in range(N_CORES):
        rows_c = np.nonzero(w == c)[0]
        if rows_c.size == 0:
            continue
        M1c = cm1[c * M_SHARD:(c + 1) * M_SHARD]
        order = np.argsort(M1c, kind="stable")
        sv = M1c[order]
        tgt = s1[rows_c]
        lo = np.searchsorted(sv, tgt, side="left")
        hi = np.searchsorted(sv, tgt, side="right")
        lane_c = lane[c * M_SHARD:(c + 1) * M_SHARD]
        erow_c = erow[c * M_SHARD:(c + 1) * M_SHARD]
        r_tail = (N_KPTS // 128 - 2) * 128
        for ri, l, h in zip(rows_c, lo, hi):
            if h == l:
                continue
            cands = [j for j in order[l:h]
                     if (ri == erow_c[j] if erow_c[j] >= 0
                         else (lane_c[j] == ri % 128 and ri < r_tail))]
            if len(cands) == 1:
                fwd_nn[ri] = cands[0] + c * M_SHARD
                found[ri] = True
            elif len(cands) > 1:
                ambig += 1
    if ambig:
        print(f"WARNING: {ambig} ambiguous column-max value collisions")

    mutual = fwd_ok & found & bck_ok[fwd_nn]

    indices0 = np.where(mutual, fwd_nn, -1)[None, :].astype(np.int32)
    mscores0 = (indices0 > 0).astype(np.int32)
    matches1 = np.full((1, M_KPTS), -1, dtype=np.int32)
    mscores1 = np.zeros((1, M_KPTS), dtype=np.float32)
    return indices0, matches1, mscores0, mscores1


def kernel(descriptors0, descriptors1, keypoints0, keypoints1):
    results = run_device(descriptors0, descriptors1)
    return postprocess(results)
